# revision 27
# baseline (speedup 1.0000x reference)
"""Bass/Trainium2 kernel for batched GNN message passing:
    out[b, d, n] = sum_m adj[b, n, m] * x[b, d, m]
B=2, D=3072, N=8192, fp32 in/out.

Sharding: 8 cores, core c -> (b = c//4, n-quarter = c%4). Each core computes
C[3072, 2048] = X[b] @ A[b, quarter, :].T with contraction m = 8192.

Strategy (bf16, zero on-chip transposes, zero DRAM partials):
- Host prepacks both operands transposed + tiled so every DMA is contiguous
  and every matmul operand is already in [contraction-on-partitions] layout.
  bf16 rounding gives rel err ~2e-3 vs the 2e-2 gate (measured on the real
  seeded inputs).
- Per core: 4 n-slabs of 512 cols. Per slab, the full-contraction adj panel
  [128k x 64mc x 512n] (64 KiB/partition) is SBUF-resident (double-buffered
  across slabs -> no PE stall at slab swap). For each of 24 d-blocks, one
  PSUM bank accumulates out[128d, 512n] over all 64 mc chunks in a dense
  back-to-back matmul stream (LDWEIGHTS hides in the PE reorder window, HAM
  stays warm). X d-block strips re-stream per slab (4x50MB, hidden under
  compute).
- Evict: PSUM -> VectorE copy -> SBUF -> DMA out. Panel DMAs issue from the
  ScalarE HWDGE queue so they prefetch ahead of the x/out Sync-queue traffic.
"""

import sys
from contextlib import ExitStack

import numpy as np

sys.path.insert(0, "/opt/trn_rl_repo")

B = 2
D = 3072
N = 8192
NCORES = 8
NSPLIT = 4  # n-quarters per batch sample
NC = N // NSPLIT  # 2048 columns of out per core

P = 128
NDB = D // P  # 24 d-blocks
NMC = N // P  # 64 contraction chunks
NSLAB = 4  # n-slabs per core
NW = NC // NSLAB  # 512 cols per slab


def build_program():
    """Build the per-core Bass program. Returns compiled nc."""
    import concourse.mybir as mybir
    import concourse.tile as tile
    from concourse import bacc

    f32 = mybir.dt.float32
    bf16 = mybir.dt.bfloat16

    nc = bacc.Bacc(None, target_bir_lowering=False, debug=False)

    # xh[db*128 + k, mc*128 + i] = x[b][db*128 + i, mc*128 + k]  (bf16)
    xh = nc.dram_tensor("xh", [D, NMC * P], bf16, kind="ExternalInput")
    # ah[ns*128 + k, mc*512 + j] = adj[b][q*2048 + ns*512 + j, mc*128 + k]
    ah = nc.dram_tensor("ah", [NSLAB * P, NMC * NW], bf16, kind="ExternalInput")
    out_ext = nc.dram_tensor("out", [D, NC], f32, kind="ExternalOutput")

    with tile.TileContext(nc) as tc, ExitStack() as ctx:
        panel_pool = ctx.enter_context(tc.tile_pool(name="panel", bufs=2))
        # x strips are half-tiles (mc 0..31 / 32..63, 8KB/part): the startup
        # interleave holds 6 of 9 bufs (vs 3 of 4 full tiles), and each half
        # frees right after its last matmul, so the post-interleave d-blocks'
        # DMAs are not starved of buffers.
        x_pool = ctx.enter_context(tc.tile_pool(name="xp", bufs=9))
        out_pool = ctx.enter_context(tc.tile_pool(name="outp", bufs=3))
        acc_psum = ctx.enter_context(tc.tile_pool(name="accp", bufs=4, space="PSUM"))

        # DMAs are split into pieces: Tile tracks sub-range deps, so matmuls
        # start as soon as the first chunk lands instead of stalling ~30us on
        # the full panel transfer. Tensor-engine instructions execute in
        # program order, so emission order is the PE schedule.

        HMC = NMC // 2  # mc chunks per x half-tile

        def load_x_half(db, half, piece_mcs, name="xs"):
            """piece_mcs: mc-chunk counts per DMA piece (sums to HMC)."""
            xs = x_pool.tile([P, HMC * P], bf16, tag="xs", name=name)
            base = half * HMC
            lo = 0
            for n in piece_mcs:
                nc.sync.dma_start(
                    out=xs[:, lo * P : (lo + n) * P],
                    in_=xh[db * P : (db + 1) * P, (base + lo) * P : (base + lo + n) * P],
                )
                lo += n
            return xs

        def mm_group(acc, halves, panel, mcs):
            for mc in mcs:
                xs = halves[mc // HMC]
                col = mc % HMC
                nc.tensor.matmul(
                    acc[:],
                    xs[:, col * P : (col + 1) * P],
                    panel[:, mc * NW : (mc + 1) * NW],
                    start=(mc == 0),
                    stop=(mc == NMC - 1),
                )

        def evict(acc, db, ns):
            osb = out_pool.tile([P, NW], f32, tag="osb")
            nc.vector.tensor_copy(out=osb[:], in_=acc[:])
            # GpSimd SWDGE queue: keeps the sync ring free for x pieces (the
            # scalar ring would head-of-line-block the panel prefetch FIFO).
            nc.gpsimd.dma_start(
                out=out_ext[db * P : (db + 1) * P, ns * NW : (ns + 1) * NW],
                in_=osb[:],
            )

        for ns in range(NSLAB):
            panel = panel_pool.tile([P, NMC * NW], bf16, tag="panel")
            # ScalarE HWDGE queue: prefetches ahead of the sync-queue traffic.
            # Graduated piece sizes (in mc chunks) for slab 0 so the first
            # matmuls start within a couple of us.
            piece_mcs = [1, 1, 2, 4, 8, 8, 8, 8, 8, 8, 8] if ns == 0 else [8] * 8

            if ns == 0:
                # Startup: panel-0 streams in at HBM pace (~25us), slower than
                # one d-block's matmuls (13.7us). Interleave the first three
                # d-blocks piece-by-piece so the PE stays busy throughout
                # (~41us of matmul work against ~41us of startup DMA).
                NI = 3
                lo = 0
                for n in piece_mcs:
                    nc.scalar.dma_start(
                        out=panel[:, lo * NW : (lo + n) * NW],
                        in_=ah[ns * P : (ns + 1) * P, lo * NW : (lo + n) * NW],
                    )
                    lo += n
                # x pieces emitted round-robin across the three tiles so
                # every tile's first chunk lands before any tile's bulk
                los = [
                    x_pool.tile([P, HMC * P], bf16, tag="xs", name=f"xlo{i}")
                    for i in range(NI)
                ]
                x_piece_mcs = [1, 2, 5, 8, 16]
                lo = 0
                for n in x_piece_mcs:
                    for db in range(NI):
                        nc.sync.dma_start(
                            out=los[db][:, lo * P : (lo + n) * P],
                            in_=xh[db * P : (db + 1) * P, lo * P : (lo + n) * P],
                        )
                    lo += n
                his = [
                    load_x_half(db, 1, [HMC], name=f"xhi{db}") for db in range(NI)
                ]
                xss = list(zip(los, his))
                accs = [
                    acc_psum.tile([P, NW], f32, tag="acc", name=f"acc{i}")
                    for i in range(NI)
                ]
                lo = 0
                for n in piece_mcs:
                    for db in range(NI):
                        mm_group(accs[db], xss[db], panel, range(lo, lo + n))
                    lo += n
                for db in range(NI):
                    evict(accs[db], db, ns)
                rest = range(NI, NDB)
            else:
                lo = 0
                for n in piece_mcs:
                    nc.scalar.dma_start(
                        out=panel[:, lo * NW : (lo + n) * NW],
                        in_=ah[ns * P : (ns + 1) * P, lo * NW : (lo + n) * NW],
                    )
                    lo += n
                rest = range(NDB)

            for db in rest:
                xlo = load_x_half(db, 0, [HMC], name="xlo")
                xhi = load_x_half(db, 1, [HMC], name="xhi")
                acc = acc_psum.tile([P, NW], f32, tag="acc")
                mm_group(acc, (xlo, xhi), panel, range(NMC))
                evict(acc, db, ns)

    nc.compile()
    return nc


_NC_CACHE = {}


def _get_program():
    if "nc" not in _NC_CACHE:
        _NC_CACHE["nc"] = build_program()
    return _NC_CACHE["nc"]


def prepare_in_maps(x: np.ndarray, adj: np.ndarray) -> list:
    """Host-side prepack: transpose + tile + bf16-cast both operands."""
    import ml_dtypes

    bf16 = ml_dtypes.bfloat16

    xh_by_b = []
    for b in range(B):
        # [D, M] -> XT [M, D] bf16 -> [mc, k, db, i] -> [db, k, mc, i]
        xt = x[b].T.astype(bf16)  # [8192, 3072] contiguous copy
        xh = (
            xt.reshape(NMC, P, NDB, P)
            .transpose(2, 1, 0, 3)
            .reshape(D, NMC * P)
        )
        xh_by_b.append(np.ascontiguousarray(xh))

    in_maps = []
    for c in range(NCORES):
        b, q = divmod(c, NSPLIT)
        a = adj[b, q * NC : (q + 1) * NC, :].astype(bf16)  # [2048, 8192]
        # [ns, j, mc, k] -> [ns, k, mc, j]
        ah = (
            a.reshape(NSLAB, NW, NMC, P)
            .transpose(0, 3, 2, 1)
            .reshape(NSLAB * P, NMC * NW)
        )
        in_maps.append({"xh": xh_by_b[b], "ah": np.ascontiguousarray(ah)})
    return in_maps


def kernel(x: np.ndarray, adj: np.ndarray) -> np.ndarray:
    """Full inputs in, full output out. x [B,D,N] f32, adj [B,N,N] f32."""
    from concourse.bass_utils import run_bass_kernel_spmd

    assert x.shape == (B, D, N) and adj.shape == (B, N, N)
    nc = _get_program()
    in_maps = prepare_in_maps(np.asarray(x), np.asarray(adj))

    res = run_bass_kernel_spmd(nc, in_maps, core_ids=list(range(NCORES)))
    out = np.empty((B, D, N), dtype=np.float32)
    for c in range(NCORES):
        b, q = divmod(c, NSPLIT)
        out[b, :, q * NC : (q + 1) * NC] = res.results[c]["out"]
    return out


# revision 28
# speedup vs baseline: 1.0244x; 1.0244x over previous
"""Bass/Trainium2 kernel for batched GNN message passing:
    out[b, d, n] = sum_m adj[b, n, m] * x[b, d, m]
B=2, D=3072, N=8192, fp32 in/out.

Sharding: 8 cores, core c -> (b = c//4, n-quarter = c%4). Each core computes
C[3072, 2048] = X[b] @ A[b, quarter, :].T with contraction m = 8192.

Strategy (bf16, zero on-chip transposes, zero DRAM partials):
- Host prepacks both operands transposed + tiled so every DMA is contiguous
  and every matmul operand is already in [contraction-on-partitions] layout.
  bf16 rounding gives rel err ~2e-3 vs the 2e-2 gate (measured on the real
  seeded inputs).
- Per core: 4 n-slabs of 512 cols. Per slab, the full-contraction adj panel
  [128k x 64mc x 512n] (64 KiB/partition) is SBUF-resident (double-buffered
  across slabs -> no PE stall at slab swap). For each of 24 d-blocks, one
  PSUM bank accumulates out[128d, 512n] over all 64 mc chunks in a dense
  back-to-back matmul stream (LDWEIGHTS hides in the PE reorder window, HAM
  stays warm). X d-block strips re-stream per slab (4x50MB, hidden under
  compute).
- Evict: PSUM -> VectorE copy -> SBUF -> DMA out. Panel DMAs issue from the
  ScalarE HWDGE queue so they prefetch ahead of the x/out Sync-queue traffic.
"""

import sys
from contextlib import ExitStack

import numpy as np

sys.path.insert(0, "/opt/trn_rl_repo")

B = 2
D = 3072
N = 8192
NCORES = 8
NSPLIT = 4  # n-quarters per batch sample
NC = N // NSPLIT  # 2048 columns of out per core

P = 128
NDB = D // P  # 24 d-blocks
NMC = N // P  # 64 contraction chunks
NSLAB = 4  # n-slabs per core
NW = NC // NSLAB  # 512 cols per slab


def build_program():
    """Build the per-core Bass program. Returns compiled nc."""
    import concourse.mybir as mybir
    import concourse.tile as tile
    from concourse import bacc

    f32 = mybir.dt.float32
    bf16 = mybir.dt.bfloat16

    nc = bacc.Bacc(None, target_bir_lowering=False, debug=False)

    # xh[db*128 + k, mc*128 + i] = x[b][db*128 + i, mc*128 + k]  (bf16)
    xh = nc.dram_tensor("xh", [D, NMC * P], bf16, kind="ExternalInput")
    # ah[ns*128 + k, mc*512 + j] = adj[b][q*2048 + ns*512 + j, mc*128 + k]
    ah = nc.dram_tensor("ah", [NSLAB * P, NMC * NW], bf16, kind="ExternalInput")
    out_ext = nc.dram_tensor("out", [D, NC], f32, kind="ExternalOutput")

    with tile.TileContext(nc) as tc, ExitStack() as ctx:
        panel_pool = ctx.enter_context(tc.tile_pool(name="panel", bufs=2))
        # x strips are half-tiles (mc 0..31 / 32..63, 8KB/part): the startup
        # interleave holds 6 of 9 bufs (vs 3 of 4 full tiles), and each half
        # frees right after its last matmul, so the post-interleave d-blocks'
        # DMAs are not starved of buffers.
        x_pool = ctx.enter_context(tc.tile_pool(name="xp", bufs=9))
        out_pool = ctx.enter_context(tc.tile_pool(name="outp", bufs=3))
        acc_psum = ctx.enter_context(tc.tile_pool(name="accp", bufs=4, space="PSUM"))

        # DMAs are split into pieces: Tile tracks sub-range deps, so matmuls
        # start as soon as the first chunk lands instead of stalling ~30us on
        # the full panel transfer. Tensor-engine instructions execute in
        # program order, so emission order is the PE schedule.

        HMC = NMC // 2  # mc chunks per x half-tile

        def load_x_half(db, half, piece_mcs, name="xs"):
            """piece_mcs: mc-chunk counts per DMA piece (sums to HMC)."""
            xs = x_pool.tile([P, HMC * P], bf16, tag="xs", name=name)
            base = half * HMC
            lo = 0
            for n in piece_mcs:
                nc.sync.dma_start(
                    out=xs[:, lo * P : (lo + n) * P],
                    in_=xh[db * P : (db + 1) * P, (base + lo) * P : (base + lo + n) * P],
                )
                lo += n
            return xs

        def mm_group(acc, halves, panel, mcs):
            for mc in mcs:
                xs = halves[mc // HMC]
                col = mc % HMC
                nc.tensor.matmul(
                    acc[:],
                    xs[:, col * P : (col + 1) * P],
                    panel[:, mc * NW : (mc + 1) * NW],
                    start=(mc == 0),
                    stop=(mc == NMC - 1),
                )

        def evict(acc, db, ns, split=1):
            """split>1 pipelines copy/DMA pieces -- used for the final tile so
            the end-of-kernel barrier waits on a smaller last DMA."""
            w = NW // split
            for i in range(split):
                osb = out_pool.tile([P, w], f32, tag="osb", name=f"osb{i}")
                nc.vector.tensor_copy(out=osb[:], in_=acc[:, i * w : (i + 1) * w])
                nc.sync.dma_start(
                    out=out_ext[
                        db * P : (db + 1) * P,
                        ns * NW + i * w : ns * NW + (i + 1) * w,
                    ],
                    in_=osb[:],
                )

        for ns in range(NSLAB):
            panel = panel_pool.tile([P, NMC * NW], bf16, tag="panel")
            # ScalarE HWDGE queue: prefetches ahead of the sync-queue traffic.
            # Graduated piece sizes (in mc chunks) for slab 0 so the first
            # matmuls start within a couple of us.
            piece_mcs = [1, 1, 2, 4, 8, 8, 8, 8, 8, 8, 8] if ns == 0 else [8] * 8

            if ns == 0:
                # Startup: panel-0 streams in at HBM pace (~25us), slower than
                # one d-block's matmuls (13.7us). Interleave the first three
                # d-blocks piece-by-piece so the PE stays busy throughout
                # (~41us of matmul work against ~41us of startup DMA).
                NI = 3
                lo = 0
                for n in piece_mcs:
                    nc.scalar.dma_start(
                        out=panel[:, lo * NW : (lo + n) * NW],
                        in_=ah[ns * P : (ns + 1) * P, lo * NW : (lo + n) * NW],
                    )
                    lo += n
                # x pieces emitted round-robin across the three tiles so
                # every tile's first chunk lands before any tile's bulk
                los = [
                    x_pool.tile([P, HMC * P], bf16, tag="xs", name=f"xlo{i}")
                    for i in range(NI)
                ]
                x_piece_mcs = [1, 2, 5, 8, 16]
                lo = 0
                for n in x_piece_mcs:
                    for db in range(NI):
                        nc.sync.dma_start(
                            out=los[db][:, lo * P : (lo + n) * P],
                            in_=xh[db * P : (db + 1) * P, lo * P : (lo + n) * P],
                        )
                    lo += n
                his = [
                    load_x_half(db, 1, [HMC], name=f"xhi{db}") for db in range(NI)
                ]
                xss = list(zip(los, his))
                accs = [
                    acc_psum.tile([P, NW], f32, tag="acc", name=f"acc{i}")
                    for i in range(NI)
                ]
                lo = 0
                for n in piece_mcs:
                    for db in range(NI):
                        mm_group(accs[db], xss[db], panel, range(lo, lo + n))
                    lo += n
                for db in range(NI):
                    evict(accs[db], db, ns)
                rest = range(NI, NDB)
            else:
                lo = 0
                for n in piece_mcs:
                    nc.scalar.dma_start(
                        out=panel[:, lo * NW : (lo + n) * NW],
                        in_=ah[ns * P : (ns + 1) * P, lo * NW : (lo + n) * NW],
                    )
                    lo += n
                rest = range(NDB)

            for db in rest:
                xlo = load_x_half(db, 0, [HMC], name="xlo")
                xhi = load_x_half(db, 1, [HMC], name="xhi")
                acc = acc_psum.tile([P, NW], f32, tag="acc")
                mm_group(acc, (xlo, xhi), panel, range(NMC))
                evict(acc, db, ns)

    nc.compile()
    return nc


_NC_CACHE = {}


def _get_program():
    if "nc" not in _NC_CACHE:
        _NC_CACHE["nc"] = build_program()
    return _NC_CACHE["nc"]


def prepare_in_maps(x: np.ndarray, adj: np.ndarray) -> list:
    """Host-side prepack: transpose + tile + bf16-cast both operands."""
    import ml_dtypes

    bf16 = ml_dtypes.bfloat16

    xh_by_b = []
    for b in range(B):
        # [D, M] -> XT [M, D] bf16 -> [mc, k, db, i] -> [db, k, mc, i]
        xt = x[b].T.astype(bf16)  # [8192, 3072] contiguous copy
        xh = (
            xt.reshape(NMC, P, NDB, P)
            .transpose(2, 1, 0, 3)
            .reshape(D, NMC * P)
        )
        xh_by_b.append(np.ascontiguousarray(xh))

    in_maps = []
    for c in range(NCORES):
        b, q = divmod(c, NSPLIT)
        a = adj[b, q * NC : (q + 1) * NC, :].astype(bf16)  # [2048, 8192]
        # [ns, j, mc, k] -> [ns, k, mc, j]
        ah = (
            a.reshape(NSLAB, NW, NMC, P)
            .transpose(0, 3, 2, 1)
            .reshape(NSLAB * P, NMC * NW)
        )
        in_maps.append({"xh": xh_by_b[b], "ah": np.ascontiguousarray(ah)})
    return in_maps


def kernel(x: np.ndarray, adj: np.ndarray) -> np.ndarray:
    """Full inputs in, full output out. x [B,D,N] f32, adj [B,N,N] f32."""
    from concourse.bass_utils import run_bass_kernel_spmd

    assert x.shape == (B, D, N) and adj.shape == (B, N, N)
    nc = _get_program()
    in_maps = prepare_in_maps(np.asarray(x), np.asarray(adj))

    res = run_bass_kernel_spmd(nc, in_maps, core_ids=list(range(NCORES)))
    out = np.empty((B, D, N), dtype=np.float32)
    for c in range(NCORES):
        b, q = divmod(c, NSPLIT)
        out[b, :, q * NC : (q + 1) * NC] = res.results[c]["out"]
    return out


# revision 29
# speedup vs baseline: 1.0283x; 1.0037x over previous
"""Bass/Trainium2 kernel for batched GNN message passing:
    out[b, d, n] = sum_m adj[b, n, m] * x[b, d, m]
B=2, D=3072, N=8192, fp32 in/out.

Sharding: 8 cores, core c -> (b = c//4, n-quarter = c%4). Each core computes
C[3072, 2048] = X[b] @ A[b, quarter, :].T with contraction m = 8192.

Strategy (bf16, zero on-chip transposes, zero DRAM partials):
- Host prepacks both operands transposed + tiled so every DMA is contiguous
  and every matmul operand is already in [contraction-on-partitions] layout.
  bf16 rounding gives rel err ~2e-3 vs the 2e-2 gate (measured on the real
  seeded inputs).
- Per core: 4 n-slabs of 512 cols. Per slab, the full-contraction adj panel
  [128k x 64mc x 512n] (64 KiB/partition) is SBUF-resident (double-buffered
  across slabs -> no PE stall at slab swap). For each of 24 d-blocks, one
  PSUM bank accumulates out[128d, 512n] over all 64 mc chunks in a dense
  back-to-back matmul stream (LDWEIGHTS hides in the PE reorder window, HAM
  stays warm). X d-block strips re-stream per slab (4x50MB, hidden under
  compute).
- Evict: PSUM -> VectorE copy -> SBUF -> DMA out. Panel DMAs issue from the
  ScalarE HWDGE queue so they prefetch ahead of the x/out Sync-queue traffic.
"""

import sys
from contextlib import ExitStack

import numpy as np

sys.path.insert(0, "/opt/trn_rl_repo")

B = 2
D = 3072
N = 8192
NCORES = 8
NSPLIT = 4  # n-quarters per batch sample
NC = N // NSPLIT  # 2048 columns of out per core

P = 128
NDB = D // P  # 24 d-blocks
NMC = N // P  # 64 contraction chunks
NSLAB = 4  # n-slabs per core
NW = NC // NSLAB  # 512 cols per slab


def build_program():
    """Build the per-core Bass program. Returns compiled nc."""
    import concourse.mybir as mybir
    import concourse.tile as tile
    from concourse import bacc

    f32 = mybir.dt.float32
    bf16 = mybir.dt.bfloat16

    nc = bacc.Bacc(None, target_bir_lowering=False, debug=False)

    # xh[db*128 + k, mc*128 + i] = x[b][db*128 + i, mc*128 + k]  (bf16)
    xh = nc.dram_tensor("xh", [D, NMC * P], bf16, kind="ExternalInput")
    # ah[ns*128 + k, mc*512 + j] = adj[b][q*2048 + ns*512 + j, mc*128 + k]
    ah = nc.dram_tensor("ah", [NSLAB * P, NMC * NW], bf16, kind="ExternalInput")
    out_ext = nc.dram_tensor("out", [D, NC], f32, kind="ExternalOutput")

    with tile.TileContext(nc) as tc, ExitStack() as ctx:
        panel_pool = ctx.enter_context(tc.tile_pool(name="panel", bufs=2))
        # x strips are half-tiles (mc 0..31 / 32..63, 8KB/part): the startup
        # interleave holds 6 of 9 bufs (vs 3 of 4 full tiles), and each half
        # frees right after its last matmul, so the post-interleave d-blocks'
        # DMAs are not starved of buffers.
        x_pool = ctx.enter_context(tc.tile_pool(name="xp", bufs=9))
        out_pool = ctx.enter_context(tc.tile_pool(name="outp", bufs=3))
        acc_psum = ctx.enter_context(tc.tile_pool(name="accp", bufs=4, space="PSUM"))

        # DMAs are split into pieces: Tile tracks sub-range deps, so matmuls
        # start as soon as the first chunk lands instead of stalling ~30us on
        # the full panel transfer. Tensor-engine instructions execute in
        # program order, so emission order is the PE schedule.

        HMC = NMC // 2  # mc chunks per x half-tile

        def load_x_half(db, half, piece_mcs, name="xs"):
            """piece_mcs: mc-chunk counts per DMA piece (sums to HMC)."""
            xs = x_pool.tile([P, HMC * P], bf16, tag="xs", name=name)
            base = half * HMC
            lo = 0
            for n in piece_mcs:
                nc.sync.dma_start(
                    out=xs[:, lo * P : (lo + n) * P],
                    in_=xh[db * P : (db + 1) * P, (base + lo) * P : (base + lo + n) * P],
                )
                lo += n
            return xs

        def mm_group(acc, halves, panel, mcs):
            for mc in mcs:
                xs = halves[mc // HMC]
                col = mc % HMC
                nc.tensor.matmul(
                    acc[:],
                    xs[:, col * P : (col + 1) * P],
                    panel[:, mc * NW : (mc + 1) * NW],
                    start=(mc == 0),
                    stop=(mc == NMC - 1),
                )

        def evict(acc, db, ns, split=1):
            """split>1 pipelines copy/DMA pieces -- used for the final tile so
            the end-of-kernel barrier waits on a smaller last DMA."""
            w = NW // split
            for i in range(split):
                osb = out_pool.tile([P, w], f32, tag="osb", name=f"osb{i}")
                nc.vector.tensor_copy(out=osb[:], in_=acc[:, i * w : (i + 1) * w])
                nc.sync.dma_start(
                    out=out_ext[
                        db * P : (db + 1) * P,
                        ns * NW + i * w : ns * NW + (i + 1) * w,
                    ],
                    in_=osb[:],
                )

        for ns in range(NSLAB):
            panel = panel_pool.tile([P, NMC * NW], bf16, tag="panel")
            # ScalarE HWDGE queue: prefetches ahead of the sync-queue traffic.
            # Graduated piece sizes (in mc chunks) for slab 0 so the first
            # matmuls start within a couple of us.
            piece_mcs = [1, 1, 2, 4, 8, 8, 8, 8, 8, 8, 8] if ns == 0 else [8] * 8

            if ns == 0:
                # Startup: panel-0 streams in at HBM pace (~25us), slower than
                # one d-block's matmuls (13.7us). Interleave the first three
                # d-blocks piece-by-piece so the PE stays busy throughout
                # (~41us of matmul work against ~41us of startup DMA).
                NI = 3
                lo = 0
                for n in piece_mcs:
                    nc.scalar.dma_start(
                        out=panel[:, lo * NW : (lo + n) * NW],
                        in_=ah[ns * P : (ns + 1) * P, lo * NW : (lo + n) * NW],
                    )
                    lo += n
                # x pieces emitted round-robin across the three tiles so
                # every tile's first chunk lands before any tile's bulk
                # db3 joins the interleave for its lo half only ("3.5-way"):
                # adds 6.8us of startup matmul work without holding an 8th
                # buffer through the whole interleave.
                los = [
                    x_pool.tile([P, HMC * P], bf16, tag="xs", name=f"xlo{i}")
                    for i in range(NI + 1)
                ]
                x_piece_mcs = [1, 2, 5, 8, 16]
                lo = 0
                for n in x_piece_mcs:
                    for db in range(NI + 1):
                        nc.sync.dma_start(
                            out=los[db][:, lo * P : (lo + n) * P],
                            in_=xh[db * P : (db + 1) * P, lo * P : (lo + n) * P],
                        )
                    lo += n
                his = [
                    load_x_half(db, 1, [HMC], name=f"xhi{db}") for db in range(NI)
                ]
                xss = list(zip(los, his))
                accs = [
                    acc_psum.tile([P, NW], f32, tag="acc", name=f"acc{i}")
                    for i in range(NI + 1)
                ]
                lo = 0
                for n in piece_mcs:
                    for db in range(NI):
                        mm_group(accs[db], xss[db], panel, range(lo, lo + n))
                    if lo < HMC:  # pieces 0..6 cover mc 0..31 exactly
                        mm_group(accs[NI], (los[NI], None), panel, range(lo, lo + n))
                    lo += n
                for db in range(NI):
                    evict(accs[db], db, ns)
                # db3 tail: hi half after the piece loop
                xhi3 = load_x_half(NI, 1, [HMC], name="xhi3")
                mm_group(accs[NI], (los[NI], xhi3), panel, range(HMC, NMC))
                evict(accs[NI], NI, ns)
                rest = range(NI + 1, NDB)
            else:
                lo = 0
                for n in piece_mcs:
                    nc.scalar.dma_start(
                        out=panel[:, lo * NW : (lo + n) * NW],
                        in_=ah[ns * P : (ns + 1) * P, lo * NW : (lo + n) * NW],
                    )
                    lo += n
                rest = range(NDB)

            for db in rest:
                xlo = load_x_half(db, 0, [HMC], name="xlo")
                xhi = load_x_half(db, 1, [HMC], name="xhi")
                acc = acc_psum.tile([P, NW], f32, tag="acc")
                mm_group(acc, (xlo, xhi), panel, range(NMC))
                evict(acc, db, ns)

    nc.compile()
    return nc


_NC_CACHE = {}


def _get_program():
    if "nc" not in _NC_CACHE:
        _NC_CACHE["nc"] = build_program()
    return _NC_CACHE["nc"]


def prepare_in_maps(x: np.ndarray, adj: np.ndarray) -> list:
    """Host-side prepack: transpose + tile + bf16-cast both operands."""
    import ml_dtypes

    bf16 = ml_dtypes.bfloat16

    xh_by_b = []
    for b in range(B):
        # [D, M] -> XT [M, D] bf16 -> [mc, k, db, i] -> [db, k, mc, i]
        xt = x[b].T.astype(bf16)  # [8192, 3072] contiguous copy
        xh = (
            xt.reshape(NMC, P, NDB, P)
            .transpose(2, 1, 0, 3)
            .reshape(D, NMC * P)
        )
        xh_by_b.append(np.ascontiguousarray(xh))

    in_maps = []
    for c in range(NCORES):
        b, q = divmod(c, NSPLIT)
        a = adj[b, q * NC : (q + 1) * NC, :].astype(bf16)  # [2048, 8192]
        # [ns, j, mc, k] -> [ns, k, mc, j]
        ah = (
            a.reshape(NSLAB, NW, NMC, P)
            .transpose(0, 3, 2, 1)
            .reshape(NSLAB * P, NMC * NW)
        )
        in_maps.append({"xh": xh_by_b[b], "ah": np.ascontiguousarray(ah)})
    return in_maps


def kernel(x: np.ndarray, adj: np.ndarray) -> np.ndarray:
    """Full inputs in, full output out. x [B,D,N] f32, adj [B,N,N] f32."""
    from concourse.bass_utils import run_bass_kernel_spmd

    assert x.shape == (B, D, N) and adj.shape == (B, N, N)
    nc = _get_program()
    in_maps = prepare_in_maps(np.asarray(x), np.asarray(adj))

    res = run_bass_kernel_spmd(nc, in_maps, core_ids=list(range(NCORES)))
    out = np.empty((B, D, N), dtype=np.float32)
    for c in range(NCORES):
        b, q = divmod(c, NSPLIT)
        out[b, :, q * NC : (q + 1) * NC] = res.results[c]["out"]
    return out
